# revision 36
# baseline (speedup 1.0000x reference)
"""Trainium2 Bass kernel for per-head-projection MHA + residual + LayerNorm.

Problem shapes (hardcoded): B=4, S=2048, E=512, H=8, DK=64, fp32.

Sharding: 8 cores, core c -> (batch b = c//2, query-half qh = c%2).
Each core computes the full transformer block for its 1024 query rows
(using the full 2048-row K/V of its batch); per-core outputs are disjoint
slices of the final [4, 2048, 512] output, no collectives.

ScalarE is the critical engine (16.8M softmax exps per core at
1 elem/cycle/lane ~ 133us; no other engine computes exp), so everything
else is organized to keep its stream dense:
  - Every matmul runs fp8e4 DoubleRow at full contraction 256
    ([128, 2, .] operands - 2 MACs/cell/cycle over all 128 PE rows).
    Q/K/V and all weights arrive host-side pre-transposed/pre-packed in
    the paired layout (E = 256m + 128i + p).
  - Q/K projection weight columns are regrouped so head h's two dk
    halves land on partitions 32*(h%4)..+32 as the two DR slices ->
    scores are [32, 2, 128] x [32, 2, 512] DR matmuls; the 4 heads of a
    quad hit disjoint PE row-groups (concurrent on hardware).
  - exp on ScalarE, fp8 out, with a folded -2 offset (cancels in
    softmax, keeps e4m3 in range). PV is DR over pairs of 128-key
    blocks; v_aug's ones column yields softmax denominators for free;
    denominator reciprocals are partition-broadcast on Pool.
  - zT is packed [128, 2, SQ] per head-quad so the final linear is two
    DR matmuls; bf_eff (bv folded through Wf) is pre-added to the
    residual rows on the host; gamma/beta broadcasts are host inputs.
  - LayerNorm: bn_stats/bn_aggr + gamma on DVE, normalize (Identity)
    and the tiny Sqrt on the otherwise-idle ScalarE, beta on Pool.
Schedule: per head, scores stream into FD-1536 exp tiles (amortizing
ScalarE's fixed 222-cycle access bubble over 1.5x more elements; exp
output is one contiguous [128, 16K] fp8 region per head so PV reads any
(key-block pair, q-chunk) stride); PV + normalization burst in the PE's
slack at each block end. K/Q/V projection chunks ride in the stream.
PSUM: 2x[128,1536] scores + 2x[128,512] shared proj/PV/final = 8 banks.
The exp stream runs gapless from ~11us to ~139us of the ~161us span.
"""

import sys

sys.path.insert(0, "/opt/trn_rl_repo")

import numpy as np

B, S, E, H, DK = 4, 2048, 512, 8, 64
NCORES = 8
SQ = (B * S) // NCORES  # 1024 query rows per core
HD = H * DK  # 512
PAIRS = H // 2
LN_EPS = 1e-5
VA_HS = 80  # per-head stride (elems, fp8) inside a v_aug slice
C_OFF = 2.0  # exp offset: exp(s/8 - C_OFF); cancels in softmax

_PROGRAM_CACHE = {}


def _build_program(repeat=1):
    from contextlib import ExitStack

    import concourse.mybir as mybir
    import concourse.tile as tile
    from concourse import bacc

    dt = mybir.dt
    f32, f32r, fp8, bf16 = dt.float32, dt.float32r, dt.float8e4, dt.bfloat16
    AF = mybir.ActivationFunctionType

    nc = bacc.Bacc("TRN2", target_bir_lowering=False, debug=False)

    # DR-layout inputs: [128, 4, seq]; E = 256m + 128i + p -> [p, 2m+i, :]
    QT_d = nc.dram_tensor("QT8", [128, 4, SQ], fp8, kind="ExternalInput").ap()
    KT_d = nc.dram_tensor("KT8", [128, 4, S], fp8, kind="ExternalInput").ap()
    VT_d = nc.dram_tensor("VT8", [128, 4, S], fp8, kind="ExternalInput").ap()
    Qn_d = nc.dram_tensor("Qn", [SQ, E], f32, kind="ExternalInput").ap()
    # weights, same DR pairing over E: [128, 4, cols]
    Wq_d = nc.dram_tensor("Wq8", [128, 4, HD], fp8, kind="ExternalInput").ap()
    Wk_d = nc.dram_tensor("Wk8", [128, 4, HD], fp8, kind="ExternalInput").ap()
    Wv_d = nc.dram_tensor("Wv8", [128, 4, HD], fp8, kind="ExternalInput").ap()
    # final linear, DR over z: [128, pairs, E]; z = 256m + 128i + p
    Wf_d = nc.dram_tensor("Wf8", [128, PAIRS, E], fp8, kind="ExternalInput").ap()
    bq_d = nc.dram_tensor("bq_g", [128, 4], f32, kind="ExternalInput").ap()
    bk_d = nc.dram_tensor("bk_g", [128, 4], f32, kind="ExternalInput").ap()
    gab_d = nc.dram_tensor("gab_b", [128, E], bf16, kind="ExternalInput").ap()
    beb_d = nc.dram_tensor("beb_b", [128, E], bf16, kind="ExternalInput").ap()
    Out_d = nc.dram_tensor("Out", [SQ, E], f32, kind="ExternalOutput").ap()

    with tile.TileContext(nc) as tc:
        for rep in range(repeat):
            _emit_body(
                nc, tc, ExitStack, mybir, f32, f32r, fp8, bf16, AF,
                QT_d, Qn_d, KT_d, VT_d, Wq_d, Wk_d, Wv_d, Wf_d,
                bq_d, bk_d, gab_d, beb_d, Out_d, rep,
            )

    nc.compile()
    return nc


def _emit_body(
    nc, tc, ExitStack, mybir, f32, f32r, fp8, bf16, AF,
    QT_d, Qn_d, KT_d, VT_d, Wq_d, Wk_d, Wv_d, Wf_d,
    bq_d, bk_d, gab_d, beb_d, Out_d, rep,
):
    DR = mybir.MatmulPerfMode.DoubleRow
    Alu = mybir.AluOpType

    with ExitStack() as ctx:
        const_p = ctx.enter_context(tc.tile_pool(name="const", bufs=1))
        w_p = ctx.enter_context(tc.tile_pool(name="weights", bufs=1))
        act_p = ctx.enter_context(tc.tile_pool(name="acts", bufs=1))
        vx_p = ctx.enter_context(tc.tile_pool(name="vx", bufs=4))
        exp_p = ctx.enter_context(tc.tile_pool(name="exp", bufs=2))
        rcp_p = ctx.enter_context(tc.tile_pool(name="rcp", bufs=2))
        rb_p = ctx.enter_context(tc.tile_pool(name="rb", bufs=2))
        ln_p = ctx.enter_context(tc.tile_pool(name="ln", bufs=6))
        st_p = ctx.enter_context(tc.tile_pool(name="stats", bufs=12))
        # PSUM: psA 2 x [128,1536] (scores, FD-1536 exp) = 6 banks;
        # psB 2 x [128,512] shared by proj/V/final accumulators during the
        # stream and PV accumulators at block ends = 2 banks
        psA = ctx.enter_context(tc.tile_pool(name="psA", bufs=2, space="PSUM"))
        psB = ctx.enter_context(tc.tile_pool(name="psB", bufs=2, space="PSUM"))

        # ---------- constants ----------
        eps_t = const_p.tile([128, 1], f32)
        nc.vector.memset(eps_t[:], LN_EPS)
        negc_t = const_p.tile([128, 1], f32)
        nc.vector.memset(negc_t[:], -C_OFF)

        # preload the Exp table while weights stream in
        wrm_in = const_p.tile([1, 16], f32)
        wrm_out = const_p.tile([1, 16], f32)
        nc.vector.memset(wrm_in[:], 0.0)
        nc.scalar.activation(wrm_out[:], wrm_in[:], AF.Exp)

        # ---------- weights / biases / staging ----------
        wq_a = w_p.tile([128, 4 * HD], fp8, tag="wqa", name=f"wqa_{rep}")
        wk_a = w_p.tile([128, 4 * HD], fp8, tag="wka", name=f"wka_{rep}")
        wv_a = w_p.tile([128, 4 * HD], fp8, tag="wva", name=f"wva_{rep}")
        wf_a = w_p.tile([128, PAIRS * E], fp8, tag="wfa", name=f"wfa_{rep}")
        bq_t = const_p.tile([128, 4], f32)
        bk_t = const_p.tile([128, 4], f32)
        gab = act_p.tile([128, E], bf16, tag="gab")
        beb = act_p.tile([128, E], bf16, tag="beb")

        kx_a = act_p.tile([128, 4 * S], fp8, tag="kxa", name=f"kxa_{rep}")
        qx_a = act_p.tile([128, 4 * SQ], fp8, tag="qxa", name=f"qxa_{rep}")
        qn_a = act_p.tile([128, 8 * E], f32, tag="qna", name=f"qna_{rep}")
        vxc = [
            vx_p.tile([128, 4 * 512], fp8, tag="vx", name=f"vx{sc}_{rep}")
            for sc in range(4)
        ]

        # projected activations: quad layout [32*(h%4)+p, dk-half, seq]
        qTq = [act_p.tile([128, 2 * SQ], fp8, tag=f"qT{i}", name=f"qT{i}_{rep}") for i in range(2)]
        kTq = [act_p.tile([128, 2 * S], fp8, tag=f"kT{i}", name=f"kT{i}_{rep}") for i in range(2)]
        # v_aug per tt-pair j: [128 keys, 2 kblocks, H*VA_HS] fp8
        v_aug = [
            act_p.tile([128, 2 * H * VA_HS], fp8, tag=f"vaug{j}", name=f"vaug{j}_{rep}")
            for j in range(8)
        ]
        # zT merged per head-quad m: [128, 2, SQ] fp8 with
        # (p, i) <-> z = 256m + 128i + p, z = head*64 + dk
        zT = [act_p.tile([128, 2 * SQ], fp8, tag=f"zT{m}", name=f"zT{m}_{rep}") for m in range(2)]

        # ---------- DMA queue (order = service order) ----------
        nc.sync.dma_start(wk_a[:].rearrange("p (s c) -> p s c", s=4), Wk_d)
        nc.sync.dma_start(bk_t[:], bk_d[:])
        kx3 = kx_a[:].rearrange("p (s t) -> p s t", s=4, t=S)
        nc.sync.dma_start(kx3[:, :, 0:512], KT_d[:, :, 0:512])
        nc.sync.dma_start(wq_a[:].rearrange("p (s c) -> p s c", s=4), Wq_d)
        nc.sync.dma_start(bq_t[:], bq_d[:])
        nc.sync.dma_start(qx_a[:].rearrange("p (s t) -> p s t", s=4, t=SQ), QT_d)
        nc.sync.dma_start(wv_a[:].rearrange("p (s c) -> p s c", s=4), Wv_d)

        def dma_kx(sc):
            nc.sync.dma_start(
                kx3[:, :, sc * 512 : (sc + 1) * 512], KT_d[:, :, sc * 512 : (sc + 1) * 512]
            )

        def dma_vx(sc):
            nc.sync.dma_start(
                vxc[sc][:].rearrange("p (s t) -> p s t", s=4, t=512),
                VT_d[:, :, sc * 512 : (sc + 1) * 512],
            )

        dma_vx(0)
        dma_kx(1)
        dma_vx(1)
        dma_kx(2)
        dma_vx(2)
        dma_vx(3)
        dma_kx(3)
        nc.sync.dma_start(wf_a[:].rearrange("p (s c) -> p s c", s=PAIRS), Wf_d)
        # prefetch the residual rows + LN constants for the tail
        nc.sync.dma_start(
            qn_a[:].rearrange("p (qb e) -> p qb e", qb=8, e=E),
            Qn_d.rearrange("(qb p) e -> p qb e", qb=8, p=128),
        )
        nc.sync.dma_start(gab[:], gab_d[:])
        nc.sync.dma_start(beb[:], beb_d[:])

        # ---------- emit helpers ----------
        def wsl(wa, m, g):
            # stationary [128, 2, 128]: contraction 256 over E half m,
            # weight column group g
            return wa[:].rearrange("p (s c) -> p s c", s=4)[
                :, 2 * m : 2 * m + 2, g * 128 : (g + 1) * 128
            ]

        def xsl(xa, m, lo, n, seq):
            # moving [128, 2, n] slice of a staged DR activation tile
            return xa[:].rearrange("p (s t) -> p s t", s=4, t=seq)[
                :, 2 * m : 2 * m + 2, lo : lo + n
            ]

        def proj_group(g, sc, wa, xa, seq, bias_t, dstq, eng):
            # one 128-column output group of a K/Q projection chunk
            pr = psB.tile([128, 512], f32, tag="psB", name=f"pj{dstq[g // 2].name}_{g}_{sc}")
            for m in range(2):
                nc.tensor.matmul(
                    pr[:], wsl(wa, m, g), xsl(xa, m, sc * 512, 512, seq),
                    start=(m == 0), stop=(m == 1), perf_mode=DR,
                )
            quad, i = g // 2, g % 2
            d2 = dstq[quad][:].rearrange("p (s t) -> p s t", s=2, t=seq)
            eng.tensor_scalar_add(
                d2[:, i, sc * 512 : (sc + 1) * 512], pr[:], bias_t[:, g : g + 1]
            )

        def k_group(g, sc):
            proj_group(g, sc, wk_a, kx_a, S, bk_t, kTq, nc.vector)

        def q_group(g, sc):
            proj_group(g, sc, wq_a, qx_a, SQ, bq_t, qTq, nc.vector)

        def v_chunk(sc):
            for tl in range(4):
                tt = sc * 4 + tl
                pr = psB.tile([128, 512], f32, tag="psB", name=f"vpj{tt}_{rep}")
                for m in range(2):
                    nc.tensor.matmul(
                        pr[:], xsl(vxc[sc], m, tl * 128, 128, 512),
                        wv_a[:].rearrange("p (s c) -> p s c", s=4)[:, 2 * m : 2 * m + 2, :],
                        start=(m == 0), stop=(m == 1), perf_mode=DR,
                    )
                j, par = tt // 2, tt % 2
                va = v_aug[j][:].rearrange("p (b h x) -> p b h x", b=2, h=H, x=VA_HS)
                pr3 = pr[:].rearrange("p (h d) -> p h d", h=H, d=DK)
                nc.vector.tensor_copy(va[:, par, :, 0:DK], pr3)
                nc.gpsimd.memset(va[:, par, :, DK : DK + 1], 1.0)

        def new_pv(h):
            return [
                psB.tile([DK + 1, 512], f32, tag="psB", name=f"pv{h}_{qc}_{rep}")
                for qc in range(2)
            ]

        # (tt, qc) chunks in tt-major order, packed into FD-1536 exp tiles
        CHUNKS = [(tt, qc) for tt in range(16) for qc in range(2)]
        EXP_SIZES = [1536] * 10 + [1024]

        def head_scores_exp(h, ex, interleave):
            # stream all scores + exps of head h; ex is the contiguous
            # [128, 16*SQ] fp8 output (offset = 1024*tt + 512*qc + q)
            quad, b = h // 4, h % 4
            pb = 32 * b
            k2 = kTq[quad][:].rearrange("p (s t) -> p s t", s=2, t=S)
            q2 = qTq[quad][:].rearrange("p (s t) -> p s t", s=2, t=SQ)
            pos = 0
            base = 0
            for c, size in enumerate(EXP_SIZES):
                scs = psA.tile([128, size], f32, tag="psA", name=f"s{h}_{c}_{rep}")
                for off in range(0, size, 512):
                    tt, qc = CHUNKS[pos]
                    pos += 1
                    nc.tensor.matmul(
                        scs[:, off : off + 512],
                        k2[pb : pb + 32, :, tt * 128 : (tt + 1) * 128],
                        q2[pb : pb + 32, :, qc * 512 : (qc + 1) * 512],
                        start=True, stop=True, perf_mode=DR,
                        tile_position=(pb, 0),
                    )
                nc.scalar.activation(
                    ex[:, base : base + size], scs[:],
                    AF.Exp, scale=float(DK) ** -0.5, bias=negc_t[:, 0:1],
                )
                base += size
                interleave(c)

        def head_pv(h, ex, pvs):
            ex3 = ex[:].rearrange("p (t q) -> p t q", t=16, q=SQ)
            for j in range(8):
                va = v_aug[j][:].rearrange("p (b c) -> p b c", b=2, c=H * VA_HS)
                for qc in range(2):
                    nc.tensor.matmul(
                        pvs[qc][:],
                        va[:, :, h * VA_HS : h * VA_HS + DK + 1],
                        ex3[:, 2 * j : 2 * j + 2, qc * 512 : (qc + 1) * 512],
                        start=(j == 0), stop=(j == 7),
                        perf_mode=DR,
                    )

        def norm_head(h, pvs):
            # reciprocal of denominators -> Pool broadcast -> zT = pv * recip
            # z = h*64 + d -> tile m = h//4, partition (h%2)*64 + d, slice (h//2)%2
            m, pb, sl = h // 4, (h % 2) * 64, (h // 2) % 2
            z2 = zT[m][:].rearrange("p (s t) -> p s t", s=2, t=SQ)
            rcp = rcp_p.tile([1, SQ], f32, tag="rcp", name=f"rcp{h}_{rep}")
            rb_sb = rb_p.tile([DK, SQ], f32, tag="rb", name=f"rbs{h}_{rep}")
            # qc-pipelined so the final linear can start after the first half
            for qc in range(2):
                nc.vector.reciprocal(
                    rcp[0:1, qc * 512 : (qc + 1) * 512],
                    pvs[qc][DK : DK + 1, :],
                )
                nc.gpsimd.partition_broadcast(
                    rb_sb[:, qc * 512 : (qc + 1) * 512],
                    rcp[0:1, qc * 512 : (qc + 1) * 512],
                )
                nc.vector.tensor_mul(
                    z2[pb : pb + DK, sl, qc * 512 : (qc + 1) * 512],
                    pvs[qc][0:DK, :],
                    rb_sb[:, qc * 512 : (qc + 1) * 512],
                )

        # ---------- schedule: per-head chunked scores/exp stream with a
        # PV + norm burst at each block end (PE has ample slack there) ----
        k_group(0, 0)
        k_group(1, 0)
        q_group(0, 0)
        q_group(0, 1)
        q_group(1, 0)
        q_group(1, 1)

        def make_interleave(h):
            def interleave(c):
                # c runs 0..10; scores chunk c covers key blocks ~(3c/2)
                if h == 0:
                    # K chunks 1-3 must precede the scores that read them
                    # (chunk c reads up to tt = (1536*(c+1))//1024); V chunk
                    # sc feeds the PV burst at block end only
                    if c in (1, 3, 5):
                        k_group(0, (c + 1) // 2)
                        k_group(1, (c + 1) // 2)
                    if c in (2, 4, 6, 8):
                        v_chunk(c // 2 - 1)
                elif h == 1:
                    if c < 4:
                        k_group(2, c)
                elif h == 2:
                    if c < 4:
                        k_group(3, c)
                elif h == 3:
                    if c < 2:
                        q_group(2, c)
                    elif c < 4:
                        q_group(3, c - 2)
            return interleave

        for h in range(H):
            ex = exp_p.tile([128, 16 * SQ], fp8, tag="exp", name=f"ex{h}_{rep}")
            head_scores_exp(h, ex, make_interleave(h))
            if h == H - 1:
                nc.scalar.activation(wrm_out[:], wrm_in[:], AF.Sqrt)
            pvs = new_pv(h)
            head_pv(h, ex, pvs)
            norm_head(h, pvs)

        # ---------- final linear + residual + LayerNorm ----------
        for qb in range(SQ // 128):
            f_ps = psB.tile([128, E], f32, tag="psB", name=f"f{qb}_{rep}")
            for m in range(2):
                z2 = zT[m][:].rearrange("p (s t) -> p s t", s=2, t=SQ)
                nc.tensor.matmul(
                    f_ps[:], z2[:, :, qb * 128 : (qb + 1) * 128],
                    wf_a[:].rearrange("p (s c) -> p s c", s=PAIRS)[:, 2 * m : 2 * m + 2, :],
                    start=(m == 0), stop=(m == 1), perf_mode=DR,
                )
            x = ln_p.tile([128, E], f32, tag="x")
            nc.vector.scalar_tensor_tensor(
                x[:], f_ps[:], 1.0, qn_a[:, qb * E : (qb + 1) * E], Alu.mult, Alu.add
            )
            bn6 = st_p.tile([128, 6], f32, tag="bn6")
            nc.vector.bn_stats(bn6[:], x[:])
            mv = st_p.tile([128, 2], f32, tag="mv")
            nc.vector.bn_aggr(mv[:], bn6[:])
            sd = st_p.tile([128, 1], f32, tag="sd")
            nc.scalar.activation(
                sd[:], mv[:, 1:2], AF.Sqrt, bias=eps_t[:, 0:1], scale=1.0
            )
            rstd = st_p.tile([128, 1], f32, tag="rstd")
            nc.vector.reciprocal(rstd[:], sd[:])
            nmr = st_p.tile([128, 1], f32, tag="nmr")
            nc.vector.scalar_tensor_tensor(
                nmr[:], mv[:, 0:1], -1.0, rstd[:], Alu.mult, Alu.mult
            )
            xn = ln_p.tile([128, E], bf16, tag="xn")
            nc.scalar.activation(
                xn[:], x[:], AF.Identity, bias=nmr[:, 0:1], scale=rstd[:, 0:1]
            )
            xg = ln_p.tile([128, E], bf16, tag="xg")
            nc.vector.tensor_mul(xg[:], xn[:], gab[:])
            xo = ln_p.tile([128, E], f32, tag="xo")
            nc.gpsimd.tensor_tensor(xo[:], xg[:], beb[:], Alu.add)
            nc.sync.dma_start(Out_d[qb * 128 : (qb + 1) * 128, :], xo[:])


def _get_program(repeat=1):
    key = f"nc{repeat}"
    if key not in _PROGRAM_CACHE:
        _PROGRAM_CACHE[key] = _build_program(repeat)
    return _PROGRAM_CACHE[key]


def _to_dr(xT):
    """[E, seq] -> [128, 4, seq] fp8: E = 256m + 128i + p -> [p, 2m+i, :]"""
    import ml_dtypes

    e, seq = xT.shape
    v = xT.reshape(2, 2, 128, seq)  # [m, i, p, seq]
    return np.ascontiguousarray(
        v.transpose(2, 0, 1, 3).reshape(128, 4, seq), dtype=ml_dtypes.float8_e4m3
    )


def _make_in_maps(Q, K, V, Wq, bq, Wk, bk, Wv, bv, Wf, bf, gamma, beta):
    import ml_dtypes

    f32 = np.float32
    bf16 = ml_dtypes.bfloat16

    def grouped_w(W):
        # [H, E, DK] -> [E, 4*128] with col g*128+32b+d = head h(g,b), dk(g,d)
        # g0: h 0-3 dk 0-31 | g1: h 0-3 dk 32-63 | g2: h 4-7 dk 0-31 | g3: ...
        Wg = W.reshape(2, 4, E, 2, 32)  # [hq, b, E, dkh, d]
        Wg = Wg.transpose(2, 0, 3, 1, 4).reshape(E, 4 * 128)  # [E, (hq dkh b d)]
        return np.asarray(Wg, dtype=f32)

    def grouped_b(b):
        bg = b.reshape(2, 4, 2, 32)  # [hq, b, dkh, d]
        return np.ascontiguousarray(
            bg.transpose(0, 2, 1, 3).reshape(4, 128).T, dtype=f32
        )

    Wq_r, Wk_r = _to_dr(grouped_w(Wq)), _to_dr(grouped_w(Wk))
    # V keeps plain h-major columns
    Wv_r = _to_dr(np.asarray(Wv.transpose(1, 0, 2).reshape(E, HD), dtype=f32))
    bq_r, bk_r = grouped_b(bq), grouped_b(bk)
    # final linear DR merged: [128, pairs, E]; z = 256m + 128i + p
    Wf8 = np.ascontiguousarray(
        Wf.reshape(2, 2, 128, E).transpose(2, 0, 1, 3).reshape(128, PAIRS, E),
        dtype=ml_dtypes.float8_e4m3,
    )
    # host precompute: bf_eff = bf + bv @ Wf (folded into the residual rows);
    # gamma/beta broadcast rows
    bfe = (bf + bv.reshape(HD) @ Wf).reshape(1, E)
    gab_b = np.ascontiguousarray(np.broadcast_to(gamma, (128, E)), dtype=bf16)
    beb_b = np.ascontiguousarray(np.broadcast_to(beta, (128, E)), dtype=bf16)

    KT8 = [_to_dr(np.asarray(K[b].T, dtype=f32)) for b in range(B)]
    VT8 = [_to_dr(np.asarray(V[b].T, dtype=f32)) for b in range(B)]

    in_maps = []
    for c in range(NCORES):
        b, qh = c // 2, c % 2
        Qs = Q[b, qh * SQ : (qh + 1) * SQ]
        in_maps.append(
            {
                "QT8": _to_dr(np.asarray(Qs.T, dtype=f32)),
                "Qn": np.ascontiguousarray(Qs + bfe, dtype=f32),
                "KT8": KT8[b],
                "VT8": VT8[b],
                "Wq8": Wq_r,
                "Wk8": Wk_r,
                "Wv8": Wv_r,
                "Wf8": Wf8,
                "bq_g": bq_r,
                "bk_g": bk_r,
                "gab_b": gab_b,
                "beb_b": beb_b,
            }
        )
    return in_maps


def run_spmd(in_maps, **kwargs):
    from concourse.bass_utils import run_bass_kernel_spmd

    nc = _get_program()
    return run_bass_kernel_spmd(nc, in_maps, list(range(NCORES)), **kwargs)


def kernel(**inputs) -> np.ndarray:
    in_maps = _make_in_maps(**inputs)
    res = run_spmd(in_maps)
    out = np.empty((B, S, E), np.float32)
    for c in range(NCORES):
        b, qh = c // 2, c % 2
        out[b, qh * SQ : (qh + 1) * SQ, :] = res.results[c]["Out"]
    return out


if __name__ == "__main__":
    import time

    t0 = time.time()
    _get_program()
    print(f"built ok in {time.time() - t0:.1f}s")
